# revision 20
# baseline (speedup 1.0000x reference)
"""Trainium2 Bass kernel for CustomGRULayer (T=512, B=64, I=H=512) on 8 NeuronCores.

Strategy:
  - Shard batch B=64 -> 8 per core (data parallel). Weights replicated.
  - Phase A (per core): u_g = x_shard @ U_g.T in fp16 on the PE; keep u resident
    in SBUF (fp16); accumulate per-(h,t) partial sums / sums-of-squares over the
    local batch shard.
  - Phase B: one AllReduce (add) of the BN partial stats across the 8 cores,
    then compute per-(h,t) coefficients a = gamma*rsqrt(var+eps),
    c' = a*mu - (beta + W_b).  (U biases cancel under BatchNorm; W biases fold
    into c'.)  Apply bnu = a*u - c' in place (broadcast over batch).
  - Phase C: sequential GRU scan over T=512 steps, entirely on-chip, in [H, B]
    layout (hidden on partitions).  Recurrent matmuls keep the fp16 weights as
    the PE stationary operand; state is fp32 with an fp16 shadow for matmuls.
    Hidden states staged per 64-step window, PE-transposed to natural [t*b, H]
    layout and DMA'd out as fp32.
"""

import os
import numpy as np
import ml_dtypes
from contextlib import ExitStack

import concourse.bass as bass
from concourse import bacc
import concourse.mybir as mybir
import concourse.tile as tile
from concourse.bass_utils import run_bass_kernel_spmd
from concourse.masks import make_identity

F32 = mybir.dt.float32
F16 = mybir.dt.float16
AF = mybir.ActivationFunctionType
ALU = mybir.AluOpType

T_FULL, B_FULL, I_DIM, H_DIM = 512, 64, 512, 512
N_CORES = 8
BL = B_FULL // N_CORES       # 8 local batch
EPS = 1e-5
KC = I_DIM // 128            # 4 contraction chunks
MC = H_DIM // 128            # 4 output chunks


def build_program(T=T_FULL, n_cores=N_CORES, collective=True):
    """Builds the SPMD Bass program (identical on all cores; data differs)."""
    PW_T = 64 if T >= 64 else T        # projection window (timesteps)
    NPW = T // PW_T
    WT = 64 if T >= 64 else T          # scan output window (timesteps)
    NW = T // WT
    NH = PW_T // 64 if PW_T >= 64 else 1   # n-halves per projection window
    NHT = PW_T // NH                   # timesteps per psum group (<=64)

    nc = bacc.Bacc("TRN2", target_bir_lowering=False, debug=False,
                   num_devices=n_cores)

    xT = nc.declare_dram_parameter("xT", [I_DIM, T, BL], F16, isOutput=False)
    uT = nc.declare_dram_parameter("uT", [3, I_DIM, H_DIM], F16, isOutput=False)
    wT = nc.declare_dram_parameter("wT", [3, H_DIM, H_DIM], F16, isOutput=False)
    gamma = nc.declare_dram_parameter("gamma", [H_DIM], F32, isOutput=False)
    betaWb = nc.declare_dram_parameter("betaWb", [3, H_DIM], F32, isOutput=False)
    h0T = nc.declare_dram_parameter("h0T", [H_DIM, BL], F16, isOutput=False)
    hs = nc.declare_dram_parameter("hs", [T, BL, H_DIM], F32, isOutput=True)

    with tile.TileContext(nc) as tc, ExitStack() as ctx:
        const = ctx.enter_context(tc.tile_pool(name="const", bufs=1))
        upool = ctx.enter_context(tc.tile_pool(name="u_res", bufs=1))
        acpool = ctx.enter_context(tc.tile_pool(name="ac", bufs=1))
        dram = ctx.enter_context(tc.tile_pool(name="dram", bufs=1, space="DRAM"))
        actx = ExitStack()   # phase-A/B-only pools, closed before the scan
        uw_pool = actx.enter_context(tc.tile_pool(name="uw", bufs=1))
        xw_pool = actx.enter_context(tc.tile_pool(name="xw", bufs=2))
        sq_pool = actx.enter_context(tc.tile_pool(name="sq", bufs=3))
        st_pool = actx.enter_context(tc.tile_pool(name="stats", bufs=2))
        fin_pool = actx.enter_context(tc.tile_pool(name="fin", bufs=2))
        ps_proj = actx.enter_context(tc.tile_pool(name="ps_proj", bufs=4, space="PSUM"))

        # ---- constants / weights ----
        w_sb = const.tile([128, 3, KC, H_DIM], F16)       # W_g.T  [k%128, g, kc, m]
        nc.sync.dma_start(w_sb[:], wT.rearrange("g (kc p) m -> p g kc m", p=128))
        u_w_sb = uw_pool.tile([128, 3, KC, H_DIM], F16)   # U_g.T (phase A only)
        nc.sync.dma_start(u_w_sb[:], uT.rearrange("g (kc p) m -> p g kc m", p=128))
        gamma_sb = const.tile([128, MC], F32)
        nc.sync.dma_start(gamma_sb[:], gamma.rearrange("(m p) -> p m", p=128))
        betaWb_sb = const.tile([128, 3, MC], F32)
        nc.sync.dma_start(betaWb_sb[:], betaWb.rearrange("g (m p) -> p g m", p=128))
        eps_sb = const.tile([128, 1], F32)
        nc.vector.memset(eps_sb[:], EPS)
        ident16 = const.tile([128, 128], F16)
        make_identity(nc, ident16[:])
        h0_sb = const.tile([128, MC, BL], F16)
        nc.sync.dma_start(h0_sb[:], h0T.rearrange("(m p) b -> p m b", p=128))

        # resident per-core projections (fp16), later overwritten with bnu
        u_sb = upool.tile([128, 3, MC, T, BL], F16)
        # BN coefficients a (idx 0) and c' (idx 1), fp16
        ac_sb = acpool.tile([128, 2, 3, MC, T], F16)

        xT_r = xT.rearrange("(kc p) t b -> p kc t b", p=128)

        stats_part = dram.tile([2, 3, H_DIM, T], F32)
        stats_glob = dram.tile([2, 3, H_DIM, T], F32, addr_space="Shared")

        # ================= Phase A: projections + partial stats =================
        for w in range(NPW):
            t0 = w * PW_T
            x_w = xw_pool.tile([128, KC, PW_T, BL], F16)
            nc.sync.dma_start(x_w[:], xT_r[:, :, t0:t0 + PW_T, :])
            st_w = st_pool.tile([128, 2, 3, MC, PW_T], F32, tag="stw")
            for g in range(3):
                for m in range(4):
                    for nh in range(NH):
                        ps = ps_proj.tile([128, NHT, BL], F32)
                        for k in range(KC):
                            nc.tensor.matmul(
                                ps[:],
                                u_w_sb[:, g, k, m * 128:(m + 1) * 128],
                                x_w[:, k, nh * NHT:(nh + 1) * NHT, :],
                                start=(k == 0), stop=(k == KC - 1),
                            )
                        tsl = slice(t0 + nh * NHT, t0 + (nh + 1) * NHT)
                        lsl = slice(nh * NHT, (nh + 1) * NHT)
                        # copy (cast fp16) into resident u
                        nc.scalar.activation(u_sb[:, g, m, tsl, :], ps[:], AF.Copy)
                        # partial sum over local batch
                        nc.vector.tensor_reduce(
                            out=st_w[:, 0, g, m, lsl], in_=ps[:],
                            axis=mybir.AxisListType.X, op=ALU.add)
                        # partial sum of squares
                        sq = sq_pool.tile([128, NHT, BL], F32)
                        nc.scalar.activation(sq[:], ps[:], AF.Square)
                        nc.vector.tensor_reduce(
                            out=st_w[:, 1, g, m, lsl], in_=sq[:],
                            axis=mybir.AxisListType.X, op=ALU.add)
            # stats window -> DRAM   [2, 3, H, T]
            nc.sync.dma_start(
                stats_part[:, :, :, t0:t0 + PW_T].rearrange(
                    "s g (m p) t -> p s g m t", p=128),
                st_w[:])

        # ================= Phase B: AllReduce + coefficients =================
        tc.strict_bb_all_engine_barrier()
        if collective:
            nc.gpsimd.collective_compute(
                "AllReduce", ALU.add,
                replica_groups=[list(range(n_cores))],
                ins=[stats_part[:]],
                outs=[stats_glob[:]],
            )
            stats_src = stats_glob
        else:
            stats_src = stats_part

        inv_b = 1.0 / (BL * (n_cores if collective else 1))
        for w in range(NPW):
            t0 = w * PW_T
            gs = st_pool.tile([128, 2, 3, MC, PW_T], F32, tag="stw")
            nc.sync.dma_start(
                gs[:], stats_src[:, :, :, t0:t0 + PW_T].rearrange(
                    "s g (m p) t -> p s g m t", p=128))
            for g in range(3):
                for m in range(4):
                    mean = fin_pool.tile([128, PW_T], F32, tag="fmean")
                    nc.vector.tensor_scalar_mul(
                        out=mean[:], in0=gs[:, 0, g, m, :], scalar1=inv_b)
                    e2 = fin_pool.tile([128, PW_T], F32, tag="fe2")
                    nc.vector.tensor_scalar_mul(
                        out=e2[:], in0=gs[:, 1, g, m, :], scalar1=inv_b)
                    # var = e2 - mean^2
                    m2 = fin_pool.tile([128, PW_T], F32, tag="fm2")
                    nc.vector.tensor_tensor(
                        out=m2[:], in0=mean[:], in1=mean[:], op=ALU.mult)
                    nc.vector.tensor_tensor(
                        out=e2[:], in0=e2[:], in1=m2[:], op=ALU.subtract)
                    # std = sqrt(var + eps); rstd = 1/std
                    nc.scalar.activation(e2[:], e2[:], AF.Sqrt, bias=eps_sb[:])
                    nc.vector.reciprocal(e2[:], e2[:])
                    # a = gamma * rstd  (fp16)
                    nc.vector.tensor_scalar_mul(
                        out=ac_sb[:, 0, g, m, t0:t0 + PW_T], in0=e2[:],
                        scalar1=gamma_sb[:, m:m + 1])
                    # c' = a*mean - (beta + W_b)
                    nc.vector.tensor_tensor(
                        out=m2[:], in0=ac_sb[:, 0, g, m, t0:t0 + PW_T],
                        in1=mean[:], op=ALU.mult)
                    nc.vector.tensor_scalar(
                        out=ac_sb[:, 1, g, m, t0:t0 + PW_T], in0=m2[:],
                        scalar1=betaWb_sb[:, g, m:m + 1], scalar2=None,
                        op0=ALU.subtract)

        # ================= Phase B2: apply bnu = a*u - c' (broadcast b) ========
        tc.strict_bb_all_engine_barrier()
        WTA = WT
        for w in range(NW):
            t0 = w * WTA
            eng = nc.vector if w == 0 else nc.gpsimd
            for g in range(3):
                for m in range(4):
                    usl = u_sb[:, g, m, t0:t0 + WTA, :]
                    for idx in (0, 1):
                        csl = ac_sb[:, idx, g, m, t0:t0 + WTA]
                        cb = bass.AP(tensor=csl.tensor, offset=csl.offset,
                                     ap=list(csl.ap) + [[0, BL]])
                        eng.tensor_tensor(
                            out=usl, in0=usl, in1=cb,
                            op=(ALU.mult if idx == 0 else ALU.subtract))

        actx.close()

        # ================= Phase C: the GRU scan =================
        # Step structure (r-gate first so its sigmoid/mul overlap the z matmuls;
        # BN'd projections injected into PSUM one step ahead via identity
        # matmuls during PE idle time; everything fp16 except PSUM):
        #   PE : [16 r-MMs][16 z-MMs][16 n-MMs][3 bias I-MMs for t+1]
        #   ACT: sig_r, sig_z, tanh      (all reading PSUM)
        #   DVE: rh = r*h, d = n-h, zd = z*d, h_new = h + zd
        win_pool = ctx.enter_context(tc.tile_pool(name="win", bufs=2))
        sm_pool = ctx.enter_context(tc.tile_pool(name="sm", bufs=3))
        nat_pool = ctx.enter_context(tc.tile_pool(name="nat", bufs=2))
        ps_r = ctx.enter_context(tc.tile_pool(name="ps_r", bufs=2, space="PSUM"))
        ps_z = ctx.enter_context(tc.tile_pool(name="ps_z", bufs=2, space="PSUM"))
        ps_n = ctx.enter_context(tc.tile_pool(name="ps_n", bufs=2, space="PSUM"))
        ps_tr = ctx.enter_context(tc.tile_pool(name="ps_tr", bufs=2, space="PSUM"))

        h_prev = h0_sb[:]                      # [128, MC, BL] fp16

        # HAM warm-up: ~7us of back-to-back wide matmuls un-throttle the PE
        # clock (1.2 -> 2.4 GHz); the scan's PE gaps stay < 3.4us afterwards
        # so it never re-throttles.
        ps_warm = ps_tr.tile([128, 512], F32, tag="pst")
        for i in range(20):
            nc.tensor.matmul(ps_warm[:], w_sb[:, 0, i % KC, 0:128],
                             u_sb[:, 0, 0, 0:64, :], start=True, stop=True)

        def bias_prefill(t):
            """psum <- bnu[t] via identity matmul (one MM per gate, N=MC*BL)."""
            pr = ps_r.tile([128, MC, BL], F32)
            pz = ps_z.tile([128, MC, BL], F32)
            pn = ps_n.tile([128, MC, BL], F32)
            for ps, g in ((pr, 1), (pz, 0), (pn, 2)):
                nc.tensor.matmul(ps[:], ident16[:], u_sb[:, g, :, t, :],
                                 start=True, stop=False, skip_group_check=True)
            return pr, pz, pn

        RT = WT * BL // 128                 # row-chunks per window
        TPR = 128 // BL                     # timesteps per row-chunk
        prev_win = None
        nat_cur = [None]

        def emit_out_piece(w_prev, tt):
            """Transpose+copy one [128,128] piece of the previous window's
            hidden states; one piece per step keeps ACT/PE FIFO stalls off
            the critical path."""
            rc, hc = tt // MC, tt % MC
            if hc == 0:
                nat_cur[0] = nat_pool.tile([128, MC, 128], F32, name='nat_piece', tag='nat')
            nat = nat_cur[0]
            pst = ps_tr.tile([128, 128], F16)
            nc.tensor.transpose(
                pst[:], prev_win[:, hc, rc * TPR:(rc + 1) * TPR, :], ident16[:])
            nc.scalar.activation(nat[:, hc, :], pst[:], AF.Copy)
            if hc == MC - 1:
                t_lo = w_prev * WT + rc * TPR
                nc.sync.dma_start(
                    hs[t_lo:t_lo + TPR, :, :].rearrange("t b h -> (t b) h"),
                    nat[:])

        pr_c, pz_c, pn_c = bias_prefill(0)
        for w in range(NW):
            win = win_pool.tile([128, MC, WT, BL], F16)
            for tt in range(WT):
                t = w * WT + tt
                pr, pz, pn = pr_c, pz_c, pn_c
                for m in range(4):
                    for k in range(KC):
                        nc.tensor.matmul(
                            pr[:, m, :],
                            w_sb[:, 1, k, m * 128:(m + 1) * 128],
                            h_prev[:, k, :],
                            start=False, stop=(k == KC - 1),
                            skip_group_check=True)
                r_t = sm_pool.tile([128, MC, BL], F16, tag="rt")
                nc.scalar.activation(r_t[:], pr[:], AF.Sigmoid)
                rh = sm_pool.tile([128, MC, BL], F16, tag="rh")
                nc.vector.tensor_tensor(out=rh[:], in0=r_t[:], in1=h_prev,
                                        op=ALU.mult)
                for m in range(4):
                    for k in range(KC):
                        nc.tensor.matmul(
                            pz[:, m, :],
                            w_sb[:, 0, k, m * 128:(m + 1) * 128],
                            h_prev[:, k, :],
                            start=False, stop=(k == KC - 1),
                            skip_group_check=True)
                z_t = sm_pool.tile([128, MC, BL], F16, tag="zt")
                nc.scalar.activation(z_t[:], pz[:], AF.Sigmoid)
                for m in range(4):
                    for k in range(KC):
                        nc.tensor.matmul(
                            pn[:, m, :],
                            w_sb[:, 2, k, m * 128:(m + 1) * 128],
                            rh[:, k, :],
                            start=False, stop=(k == KC - 1),
                            skip_group_check=True)
                if t + 1 < T:
                    pr_c, pz_c, pn_c = bias_prefill(t + 1)
                n_t = sm_pool.tile([128, MC, BL], F16, tag="nt")
                nc.scalar.activation(n_t[:], pn[:], AF.Tanh)
                d_t = sm_pool.tile([128, MC, BL], F16, tag="dt")
                nc.vector.tensor_tensor(out=d_t[:], in0=n_t[:], in1=h_prev,
                                        op=ALU.subtract)
                zd = sm_pool.tile([128, MC, BL], F16, tag="zd")
                nc.vector.tensor_tensor(out=zd[:], in0=z_t[:], in1=d_t[:],
                                        op=ALU.mult)
                h_new = win[:, :, tt, :]
                nc.vector.tensor_tensor(out=h_new, in0=h_prev, in1=zd[:],
                                        op=ALU.add)
                h_prev = h_new
                # previous window's output, one [128,128] piece per step
                if prev_win is not None and tt < RT * MC:
                    emit_out_piece(w - 1, tt)
            prev_win = win

        # flush the final window's output
        for tt in range(RT * MC):
            emit_out_piece(NW - 1, tt)

    return nc


def _install_ntff_shim():
    """Register the axon NTFF profiling hook (missing from this container's
    antenv) so run_bass_kernel_spmd(trace=True) can capture exec time."""
    import sys, types
    try:
        from antenv.axon_hooks import get_axon_ntff_profile_hook  # noqa: F401
        return  # already available
    except ImportError:
        pass
    try:
        from trn_agent_boot.trn_boot import _ntff_profile_via_ctypes
        hook = _ntff_profile_via_ctypes('/opt/axon/libaxon_pjrt.so')
    except Exception:
        hook = None
    mod = types.ModuleType("antenv.axon_hooks")
    mod._hook = hook
    mod.get_axon_ntff_profile_hook = lambda: mod._hook
    mod.set_axon_ntff_profile_hook = lambda h: setattr(mod, "_hook", h)
    sys.modules["antenv.axon_hooks"] = mod
    import antenv
    antenv.axon_hooks = mod
    # keep artifacts local (no remote bucket in this container)
    import concourse.bass_utils as bu
    bu.upload_artifacts = lambda tmpdir: tmpdir


_NC_CACHE = {}


def _get_program():
    key = (T_FULL, N_CORES)
    if key not in _NC_CACHE:
        nc = build_program()
        nc.finalize()
        _NC_CACHE[key] = nc
    return _NC_CACHE[key]


def kernel(x, h0, W_z_w, W_z_b, U_z_w, U_z_b, W_r_w, W_r_b, U_r_w, U_r_b,
           W_h_w, W_h_b, U_h_w, U_h_b, bn_gamma, bn_beta, **kw):
    T, B, I = x.shape
    H = h0.shape[-1]
    assert (T, B, I, H) == (T_FULL, B_FULL, I_DIM, H_DIM)

    f16 = np.float16
    xT16 = np.asarray(x).astype(f16).transpose(2, 0, 1)          # [I, T, B]
    uT_np = np.stack([np.asarray(U_z_w).T, np.asarray(U_r_w).T,
                      np.asarray(U_h_w).T]).astype(f16)          # [3, I, H]
    wT_np = np.stack([np.asarray(W_z_w).T, np.asarray(W_r_w).T,
                      np.asarray(W_h_w).T]).astype(f16)          # [3, H, H]
    gamma_np = np.asarray(bn_gamma).astype(np.float32)
    betaWb_np = np.stack([np.asarray(bn_beta) + np.asarray(W_z_b),
                          np.asarray(bn_beta) + np.asarray(W_r_b),
                          np.asarray(bn_beta) + np.asarray(W_h_b)]
                         ).astype(np.float32)                    # [3, H]
    h0T_np = np.asarray(h0)[0].T.astype(np.float16)              # [H, B]

    in_maps = []
    for c in range(N_CORES):
        sl = slice(c * BL, (c + 1) * BL)
        in_maps.append({
            "xT": np.ascontiguousarray(xT16[:, :, sl]),
            "uT": uT_np,
            "wT": wT_np,
            "gamma": gamma_np,
            "betaWb": betaWb_np,
            "h0T": np.ascontiguousarray(h0T_np[:, sl]),
        })

    nc = _get_program()
    trace = os.environ.get("GRU_TRACE", "0") == "1"
    if trace:
        _install_ntff_shim()
    res = run_bass_kernel_spmd(nc, in_maps, core_ids=list(range(N_CORES)),
                               trace=trace)
    if trace and res.exec_time_ns is not None:
        print(f"HW exec time: {res.exec_time_ns} ns")
    hs = np.concatenate([r["hs"] for r in res.results], axis=1)  # [T, B, H]
    hs = hs.astype(np.float32)
    return hs.reshape(T, B, 1, H), hs.reshape(1, T, B, H)


# revision 22
# speedup vs baseline: 1.0430x; 1.0430x over previous
"""Trainium2 Bass kernel for CustomGRULayer (T=512, B=64, I=H=512) on 8 NeuronCores.

Strategy:
  - Shard batch B=64 -> 8 per core (data parallel). Weights replicated. The only
    cross-core traffic is an AllReduce of BatchNorm partial stats.
  - BatchNorm here is per-timestep over the batch, and it is applied to the
    input projections only (u_g = x @ U_g.T), never to the recurrent state.  So
    the whole BN pipeline is windowed over T and overlapped with the sequential
    scan: while the GRU scan runs window w, window w+1's projections run in PE
    idle gaps, its stats/apply run on GpSimd, and its stats AllReduce runs on
    the collective cores.
  - Math simplifications: U biases cancel under BN; W biases and beta fold into
    the BN shift: bnu = a*u - c', a = gamma*rsqrt(var+eps),
    c' = a*mu - (beta + W_b).
  - The scan keeps everything fp16 (weights, state, bnu) except PSUM (fp32).
    Weights stay in [H,B] layout (hidden on partitions); per step:
      PE : [16 r-MMs][16 z-MMs][16 n-MMs][3 bias identity-MMs for t+1]
      ACT: sig_r, sig_z, tanh  (reading PSUM directly - the BN'd projection was
           injected into PSUM ahead of time via an identity matmul)
      DVE: rh = r*h, d = n-h, zd = z*d, h_new = h + zd
    Hidden states are staged per window and PE-transposed to the natural
    [t*b, H] layout one 128x128 piece per step, then DMA'd out as fp32.
"""

import os
import numpy as np
from contextlib import ExitStack

import concourse.bass as bass
from concourse import bacc
import concourse.mybir as mybir
import concourse.tile as tile
from concourse.bass_utils import run_bass_kernel_spmd
from concourse.masks import make_identity

F32 = mybir.dt.float32
F16 = mybir.dt.float16
AF = mybir.ActivationFunctionType
ALU = mybir.AluOpType

T_FULL, B_FULL, I_DIM, H_DIM = 512, 64, 512, 512
N_CORES = 8
BL = B_FULL // N_CORES       # 8 local batch
EPS = 1e-5
KC = I_DIM // 128            # 4 contraction chunks
MC = H_DIM // 128            # 4 output chunks


def build_program(T=T_FULL, n_cores=N_CORES, collective=True):
    """Builds the SPMD Bass program (identical on all cores; data differs)."""
    WT = 64 if T >= 64 else T          # window (timesteps)
    NW = T // WT

    nc = bacc.Bacc("TRN2", target_bir_lowering=False, debug=False,
                   num_devices=n_cores)

    xT = nc.declare_dram_parameter("xT", [I_DIM, T, BL], F16, isOutput=False)
    uT = nc.declare_dram_parameter("uT", [3, I_DIM, H_DIM], F16, isOutput=False)
    wT = nc.declare_dram_parameter("wT", [3, H_DIM, H_DIM], F16, isOutput=False)
    gamma = nc.declare_dram_parameter("gamma", [H_DIM], F32, isOutput=False)
    betaWb = nc.declare_dram_parameter("betaWb", [3, H_DIM], F32, isOutput=False)
    h0T = nc.declare_dram_parameter("h0T", [H_DIM, BL], F16, isOutput=False)
    hs = nc.declare_dram_parameter("hs", [T, BL, H_DIM], F32, isOutput=True)

    with tile.TileContext(nc) as tc, ExitStack() as ctx:
        const = ctx.enter_context(tc.tile_pool(name="const", bufs=1))
        upool = ctx.enter_context(tc.tile_pool(name="u_win", bufs=3))
        acpool = ctx.enter_context(tc.tile_pool(name="ac", bufs=1))
        dram = ctx.enter_context(tc.tile_pool(name="dram", bufs=1, space="DRAM"))
        xw_pool = ctx.enter_context(tc.tile_pool(name="xw", bufs=2))
        sq_pool = ctx.enter_context(tc.tile_pool(name="sq", bufs=2))
        st_pool = ctx.enter_context(tc.tile_pool(name="stats", bufs=2))
        fin_pool = ctx.enter_context(tc.tile_pool(name="fin", bufs=2))
        win_pool = ctx.enter_context(tc.tile_pool(name="win", bufs=2))
        sm_pool = ctx.enter_context(tc.tile_pool(name="sm", bufs=3))
        nat_pool = ctx.enter_context(tc.tile_pool(name="nat", bufs=2))
        ps_r = ctx.enter_context(tc.tile_pool(name="ps_r", bufs=2, space="PSUM"))
        ps_z = ctx.enter_context(tc.tile_pool(name="ps_z", bufs=2, space="PSUM"))
        ps_n = ctx.enter_context(tc.tile_pool(name="ps_n", bufs=2, space="PSUM"))
        ps_misc = ctx.enter_context(tc.tile_pool(name="ps_misc", bufs=2, space="PSUM"))

        # ---- constants / weights ----
        w_sb = const.tile([128, 3, KC, H_DIM], F16)       # W_g.T  [k%128, g, kc, m]
        nc.sync.dma_start(w_sb[:], wT.rearrange("g (kc p) m -> p g kc m", p=128))
        u_w_sb = const.tile([128, 3, KC, H_DIM], F16)     # U_g.T
        nc.sync.dma_start(u_w_sb[:], uT.rearrange("g (kc p) m -> p g kc m", p=128))
        gamma_sb = const.tile([128, MC], F32)
        nc.sync.dma_start(gamma_sb[:], gamma.rearrange("(m p) -> p m", p=128))
        betaWb_sb = const.tile([128, 3, MC], F32)
        nc.sync.dma_start(betaWb_sb[:], betaWb.rearrange("g (m p) -> p g m", p=128))
        eps_sb = const.tile([128, 1], F32)
        nc.vector.memset(eps_sb[:], EPS)
        ident16 = const.tile([128, 128], F16)
        make_identity(nc, ident16[:])
        h0_sb = const.tile([128, MC, BL], F16)
        nc.sync.dma_start(h0_sb[:], h0T.rearrange("(m p) b -> p m b", p=128))

        # BN coefficients a (idx 0) and c' (idx 1), fp16, all T resident
        ac_sb = acpool.tile([128, 2, 3, MC, T], F16)

        xT_r = xT.rearrange("(kc p) t b -> p kc t b", p=128)
        inv_b = 1.0 / (BL * (n_cores if collective else 1))

        # ============ window preparation: proj + stats + BN coeffs + apply ====
        def prepare_window(w, first):
            """Emit projection/stats/AllReduce/finalize/apply for window w.
            Returns the fp16 bnu tile [128, 3, MC, WT, BL] the scan reads."""
            t0 = w * WT
            x_w = xw_pool.tile([128, KC, WT, BL], F16, name=f"x_w{w}", tag="xw")
            nc.sync.dma_start(x_w[:], xT_r[:, :, t0:t0 + WT, :])
            uw_t = upool.tile([128, 3, MC, WT, BL], F16, name=f"uw{w}", tag="uw")
            st_w = st_pool.tile([128, 2, 3, MC, WT], F32, name=f"st{w}", tag="stw")
            peng = nc.vector if first else nc.gpsimd
            # free-axis reduction is DVE-only; squares/apply can go to GpSimd
            for g in range(3):
                for m in range(4):
                    ps = ps_misc.tile([128, WT, BL], F32, name=f"pp{w}_{g}_{m}",
                                      tag="pmisc")
                    for k in range(KC):
                        nc.tensor.matmul(
                            ps[:], u_w_sb[:, g, k, m * 128:(m + 1) * 128],
                            x_w[:, k, :, :], start=(k == 0), stop=(k == KC - 1))
                    # cast fp16 into the window's u tile
                    nc.scalar.activation(uw_t[:, g, m, :, :], ps[:], AF.Copy)
                    # batch-shard partial sums / sums of squares (off DVE/ACT
                    # during the scan: GpSimd)
                    usl = uw_t[:, g, m, :, :]
                    nc.vector.tensor_reduce(out=st_w[:, 0, g, m, :], in_=usl,
                                            axis=mybir.AxisListType.X, op=ALU.add)
                    sq = sq_pool.tile([128, WT, BL], F16, name=f"sq{w}_{g}_{m}",
                                      tag="sq")
                    peng.tensor_tensor(out=sq[:], in0=usl, in1=usl, op=ALU.mult)
                    nc.vector.tensor_reduce(out=st_w[:, 1, g, m, :], in_=sq[:],
                                            axis=mybir.AxisListType.X, op=ALU.add)
            sp = dram.tile([2, 3, H_DIM, WT], F32, name=f"sp{w}", tag=f"sp{w}")
            nc.sync.dma_start(
                sp[:].rearrange("s g (m p) t -> p s g m t", p=128), st_w[:])
            if collective:
                sg = dram.tile([2, 3, H_DIM, WT], F32, name=f"sg{w}",
                               tag=f"sg{w}", addr_space="Shared")
                nc.gpsimd.collective_compute(
                    "AllReduce", ALU.add,
                    replica_groups=[list(range(n_cores))],
                    ins=[sp[:]], outs=[sg[:]])
            else:
                sg = sp
            gs = st_pool.tile([128, 2, 3, MC, WT], F32, name=f"gs{w}", tag="stw")
            nc.sync.dma_start(
                gs[:], sg[:].rearrange("s g (m p) t -> p s g m t", p=128))
            # ---- finalize coefficients (small; DVE + ACT) ----
            tsl = slice(t0, t0 + WT)
            for g in range(3):
                for m in range(4):
                    mean = fin_pool.tile([128, WT], F32, tag="fmean")
                    nc.vector.tensor_scalar_mul(
                        out=mean[:], in0=gs[:, 0, g, m, :], scalar1=inv_b)
                    e2 = fin_pool.tile([128, WT], F32, tag="fe2")
                    nc.vector.tensor_scalar_mul(
                        out=e2[:], in0=gs[:, 1, g, m, :], scalar1=inv_b)
                    m2 = fin_pool.tile([128, WT], F32, tag="fm2")
                    nc.vector.tensor_tensor(out=m2[:], in0=mean[:], in1=mean[:],
                                            op=ALU.mult)
                    nc.vector.tensor_tensor(out=e2[:], in0=e2[:], in1=m2[:],
                                            op=ALU.subtract)
                    nc.scalar.activation(e2[:], e2[:], AF.Sqrt, bias=eps_sb[:])
                    nc.vector.reciprocal(e2[:], e2[:])
                    nc.vector.tensor_scalar_mul(
                        out=ac_sb[:, 0, g, m, tsl], in0=e2[:],
                        scalar1=gamma_sb[:, m:m + 1])
                    nc.vector.tensor_tensor(
                        out=m2[:], in0=ac_sb[:, 0, g, m, tsl], in1=mean[:],
                        op=ALU.mult)
                    nc.vector.tensor_scalar(
                        out=ac_sb[:, 1, g, m, tsl], in0=m2[:],
                        scalar1=betaWb_sb[:, g, m:m + 1], scalar2=None,
                        op0=ALU.subtract)
            # ---- apply bnu = a*u - c' in place (broadcast over batch) ----
            aeng = nc.vector if first else nc.gpsimd
            for g in range(3):
                for m in range(4):
                    usl = uw_t[:, g, m, :, :]
                    for idx in (0, 1):
                        csl = ac_sb[:, idx, g, m, tsl]
                        cb = bass.AP(tensor=csl.tensor, offset=csl.offset,
                                     ap=list(csl.ap) + [[0, BL]])
                        aeng.tensor_tensor(
                            out=usl, in0=usl, in1=cb,
                            op=(ALU.mult if idx == 0 else ALU.subtract))
            return uw_t

        # ================= the pipelined GRU scan =================
        h_prev = h0_sb[:]                      # [128, MC, BL] fp16

        def bias_prefill(uw_t, tt):
            """psum <- bnu[t] via identity matmul (one MM per gate, N=MC*BL)."""
            pr = ps_r.tile([128, MC, BL], F32)
            pz = ps_z.tile([128, MC, BL], F32)
            pn = ps_n.tile([128, MC, BL], F32)
            for ps, g in ((pr, 1), (pz, 0), (pn, 2)):
                nc.tensor.matmul(ps[:], ident16[:], uw_t[:, g, :, tt, :],
                                 start=True, stop=False, skip_group_check=True)
            return pr, pz, pn

        RT = WT * BL // 128                 # row-chunks per window
        TPR = 128 // BL                     # timesteps per row-chunk
        prev_win = None
        nat_cur = [None]

        def emit_out_piece(w_prev, tt):
            """Transpose+copy one [128,128] piece of the previous window's
            hidden states; one piece per step keeps ACT/PE FIFO stalls off
            the critical path."""
            rc, hc = tt // MC, tt % MC
            if hc == 0:
                nat_cur[0] = nat_pool.tile([128, MC, 128], F32,
                                           name="nat_piece", tag="nat")
            nat = nat_cur[0]
            pst = ps_misc.tile([128, 128], F16, name="pst", tag="pmisc")
            nc.tensor.transpose(
                pst[:], prev_win[:, hc, rc * TPR:(rc + 1) * TPR, :], ident16[:])
            nc.scalar.activation(nat[:, hc, :], pst[:], AF.Copy)
            if hc == MC - 1:
                t_lo = w_prev * WT + rc * TPR
                nc.sync.dma_start(
                    hs[t_lo:t_lo + TPR, :, :].rearrange("t b h -> (t b) h"),
                    nat[:])

        uw_cur = prepare_window(0, first=True)
        uw_next = None
        pr_c, pz_c, pn_c = bias_prefill(uw_cur, 0)
        for w in range(NW):
            win = win_pool.tile([128, MC, WT, BL], F16)
            for tt in range(WT):
                pr, pz, pn = pr_c, pz_c, pn_c
                for m in range(4):
                    for k in range(KC):
                        nc.tensor.matmul(
                            pr[:, m, :],
                            w_sb[:, 1, k, m * 128:(m + 1) * 128],
                            h_prev[:, k, :],
                            start=False, stop=(k == KC - 1),
                            skip_group_check=True)
                r_t = sm_pool.tile([128, MC, BL], F16, tag="rt")
                nc.scalar.activation(r_t[:], pr[:], AF.Sigmoid)
                rh = sm_pool.tile([128, MC, BL], F16, tag="rh")
                nc.vector.tensor_tensor(out=rh[:], in0=r_t[:], in1=h_prev,
                                        op=ALU.mult)
                for m in range(4):
                    for k in range(KC):
                        nc.tensor.matmul(
                            pz[:, m, :],
                            w_sb[:, 0, k, m * 128:(m + 1) * 128],
                            h_prev[:, k, :],
                            start=False, stop=(k == KC - 1),
                            skip_group_check=True)
                z_t = sm_pool.tile([128, MC, BL], F16, tag="zt")
                nc.scalar.activation(z_t[:], pz[:], AF.Sigmoid)
                for m in range(4):
                    for k in range(KC):
                        nc.tensor.matmul(
                            pn[:, m, :],
                            w_sb[:, 2, k, m * 128:(m + 1) * 128],
                            rh[:, k, :],
                            start=False, stop=(k == KC - 1),
                            skip_group_check=True)
                t_next = w * WT + tt + 1
                if t_next < T:
                    nxt = uw_cur if (tt + 1 < WT) else uw_next
                    pr_c, pz_c, pn_c = bias_prefill(nxt, (tt + 1) % WT)
                n_t = sm_pool.tile([128, MC, BL], F16, tag="nt")
                nc.scalar.activation(n_t[:], pn[:], AF.Tanh)
                d_t = sm_pool.tile([128, MC, BL], F16, tag="dt")
                nc.vector.tensor_tensor(out=d_t[:], in0=n_t[:], in1=h_prev,
                                        op=ALU.subtract)
                zd = sm_pool.tile([128, MC, BL], F16, tag="zd")
                nc.vector.tensor_tensor(out=zd[:], in0=z_t[:], in1=d_t[:],
                                        op=ALU.mult)
                h_new = win[:, :, tt, :]
                nc.vector.tensor_tensor(out=h_new, in0=h_prev, in1=zd[:],
                                        op=ALU.add)
                h_prev = h_new
                # previous window's output, one [128,128] piece per step
                if prev_win is not None and tt < RT * MC:
                    emit_out_piece(w - 1, tt)
                # next window's preparation fills engine idle time; emitted
                # right after the first step of this window so its priority
                # is below the current window's scan chain
                if tt == 0 and w + 1 < NW:
                    uw_next = prepare_window(w + 1, first=False)
            prev_win = win
            if w + 1 < NW:
                uw_cur = uw_next

        # flush the final window's output
        for tt in range(RT * MC):
            emit_out_piece(NW - 1, tt)

    return nc


def _install_ntff_shim():
    """Register the axon NTFF profiling hook (missing from this container's
    antenv) so run_bass_kernel_spmd(trace=True) can capture exec time."""
    import sys, types
    try:
        from antenv.axon_hooks import get_axon_ntff_profile_hook  # noqa: F401
        return  # already available
    except ImportError:
        pass
    try:
        from trn_agent_boot.trn_boot import _ntff_profile_via_ctypes
        hook = _ntff_profile_via_ctypes('/opt/axon/libaxon_pjrt.so')
    except Exception:
        hook = None
    mod = types.ModuleType("antenv.axon_hooks")
    mod._hook = hook
    mod.get_axon_ntff_profile_hook = lambda: mod._hook
    mod.set_axon_ntff_profile_hook = lambda h: setattr(mod, "_hook", h)
    sys.modules["antenv.axon_hooks"] = mod
    import antenv
    antenv.axon_hooks = mod
    # keep artifacts local (no remote bucket in this container)
    import concourse.bass_utils as bu
    bu.upload_artifacts = lambda tmpdir: tmpdir


_NC_CACHE = {}


def _get_program():
    key = (T_FULL, N_CORES)
    if key not in _NC_CACHE:
        nc = build_program()
        nc.finalize()
        _NC_CACHE[key] = nc
    return _NC_CACHE[key]


def kernel(x, h0, W_z_w, W_z_b, U_z_w, U_z_b, W_r_w, W_r_b, U_r_w, U_r_b,
           W_h_w, W_h_b, U_h_w, U_h_b, bn_gamma, bn_beta, **kw):
    T, B, I = x.shape
    H = h0.shape[-1]
    assert (T, B, I, H) == (T_FULL, B_FULL, I_DIM, H_DIM)

    f16 = np.float16
    xT16 = np.asarray(x).astype(f16).transpose(2, 0, 1)          # [I, T, B]
    uT_np = np.stack([np.asarray(U_z_w).T, np.asarray(U_r_w).T,
                      np.asarray(U_h_w).T]).astype(f16)          # [3, I, H]
    wT_np = np.stack([np.asarray(W_z_w).T, np.asarray(W_r_w).T,
                      np.asarray(W_h_w).T]).astype(f16)          # [3, H, H]
    gamma_np = np.asarray(bn_gamma).astype(np.float32)
    betaWb_np = np.stack([np.asarray(bn_beta) + np.asarray(W_z_b),
                          np.asarray(bn_beta) + np.asarray(W_r_b),
                          np.asarray(bn_beta) + np.asarray(W_h_b)]
                         ).astype(np.float32)                    # [3, H]
    h0T_np = np.asarray(h0)[0].T.astype(np.float16)              # [H, B]

    in_maps = []
    for c in range(N_CORES):
        sl = slice(c * BL, (c + 1) * BL)
        in_maps.append({
            "xT": np.ascontiguousarray(xT16[:, :, sl]),
            "uT": uT_np,
            "wT": wT_np,
            "gamma": gamma_np,
            "betaWb": betaWb_np,
            "h0T": np.ascontiguousarray(h0T_np[:, sl]),
        })

    nc = _get_program()
    trace = os.environ.get("GRU_TRACE", "0") == "1"
    if trace:
        _install_ntff_shim()
    res = run_bass_kernel_spmd(nc, in_maps, core_ids=list(range(N_CORES)),
                               trace=trace)
    if trace and res.exec_time_ns is not None:
        print(f"HW exec time: {res.exec_time_ns} ns")
    hs = np.concatenate([r["hs"] for r in res.results], axis=1)  # [T, B, H]
    hs = hs.astype(np.float32)
    return hs.reshape(T, B, 1, H), hs.reshape(1, T, B, H)
